# revision 76
# baseline (speedup 1.0000x reference)
"""Trainium2 Bass kernel for MultiHeadFAVORAttention.

Sharding: 8 cores, data-parallel over (batch, seq): core j owns batch b=j//4,
tokens [(j%4)*1024, (j%4+1)*1024). The only cross-token coupling (kv / ksum
reductions over S) is handled with one AllReduce per batch half over the
per-head [65, 256] kv^T-augmented matrices (ksum folded in as an extra v
column of ones).

Per-core pipeline (fp32 data, fp32r matmuls; front end in two 512-token
halves to fit SBUF), ordered so the kv AllReduces overlap the q front end:
  pass A (k, v): grouped conv -> multi-scale RFF (cos via explicit range
    reduction + ACT Sin) -> dense proj. k is produced channel-major with
    RoPE applied via a PE partition-permutation and |k|^2 rows accumulated
    with block-ones stationaries, spilled directly in the nystrom-ready
    [66, T] per-head layout; v is token-major with an appended ones column.
  k side: per head, nystrom RBF vs landmarks (|k|^2 folded via an augmented
    contraction row) -> FAVOR (|kn|^2 via augmented omega rows; the q-side
    norm cancels in num/denom up to eps=1e-6) -> kv^T; AllReduce per batch
    half issued at heads 7/15.
  pass B (q): channel-major proj + RoPE, spilled per channel chunk.
  q side: FAVOR exp -> num/denom (denominator via the augmented kv row,
    reciprocal broadcast across partitions) -> output dense.
q/k/v intermediates spill to DRAM between phases to fit SBUF.

_build(ncores, phases, repeat): `repeat` emits the whole body N times in one
NEFF (idempotent; collectives pair by emission order) so test.py can
amortize the ~2.5ms/dispatch axon RPC overhead out of the timing loop.
"""
import os

if os.environ.get("JAX_PLATFORMS", "").strip().lower() == "cpu":
    # bass2jax runs the NEFF through the axon PJRT plugin; a cpu pin would
    # hide the 8 NeuronCores from jax
    os.environ["JAX_PLATFORMS"] = ""

import numpy as np

T = 1024          # tokens per core
HF = 512          # front-end half
C = 1024
H = 16
DH = 64
M = 256           # FAVOR features
G = 3
MS = 256
NCORES = 8
E2 = 66          # padded aug width (fp32r needs even free dims)
MAGIC = float(1.5 * 2 ** 23)
TWO_PI = float(2.0 * np.pi)

_CACHE = {}


def _build(ncores, phases="fekqo", repeat=1, collectives=True):
    import concourse.bacc as bacc
    import concourse.tile as tile
    from concourse import mybir
    from contextlib import ExitStack

    f32, f32r = mybir.dt.float32, mybir.dt.float32r
    bf16 = mybir.dt.bfloat16
    AF = mybir.ActivationFunctionType
    ALU = mybir.AluOpType

    nc = bacc.Bacc("TRN2", target_bir_lowering=False, debug=False,
                   num_devices=ncores)

    # ---- inputs ----
    xT = nc.dram_tensor("xT", [C, T + 2], f32r, kind="ExternalInput")
    convw = nc.dram_tensor("convw", [3, 3, 256, C], f32r, kind="ExternalInput")
    cb = nc.dram_tensor("cb", [128, 3, 8], f32, kind="ExternalInput")
    rffw = nc.dram_tensor("rffw", [3, G, C, MS], f32r, kind="ExternalInput")
    rffb2 = nc.dram_tensor("rffb2", [128, 3, G, 2], f32, kind="ExternalInput")
    rffbr = nc.dram_tensor("rffbr", [128, 3, G, 2], f32, kind="ExternalInput")
    projw = nc.dram_tensor("projw", [3, G * MS, C], bf16, kind="ExternalInput")
    outw = nc.dram_tensor("outw", [C, C], bf16, kind="ExternalInput")
    outb = nc.dram_tensor("outb", [128, 8], f32, kind="ExternalInput")
    omegx = nc.dram_tensor("omegx", [128, M], bf16, kind="ExternalInput")
    omegq = nc.dram_tensor("omegq", [128, M], bf16, kind="ExternalInput")
    lmT = nc.dram_tensor("lmT", [E2, 128], bf16, kind="ExternalInput")
    nlm2 = nc.dram_tensor("nlm2", [128, 1], f32, kind="ExternalInput")
    nscal = nc.dram_tensor("nscal", [128, 1], f32, kind="ExternalInput")
    swp = nc.dram_tensor("swp", [128, 128], bf16, kind="ExternalInput")
    qcos = nc.dram_tensor("qcos", [128, T], bf16, kind="ExternalInput")
    qsin = nc.dram_tensor("qsin", [128, T], bf16, kind="ExternalInput")
    ident = nc.dram_tensor("ident", [128, 128], bf16, kind="ExternalInput")

    outT = nc.dram_tensor("outT", [C, T], f32, kind="ExternalOutput")

    rg = [[0, 1, 2, 3], [4, 5, 6, 7]] if ncores == NCORES else [[0]]
    sgam = [float(np.sqrt(2.0 * g)) for g in (0.5, 1.0, 2.0)]

    with tile.TileContext(nc) as tc, ExitStack() as ctx:
        cpool = ctx.enter_context(tc.tile_pool(name="const", bufs=1))
        dpool = ctx.enter_context(tc.tile_pool(name="dram", bufs=1,
                                               space="DRAM"))

        # DRAM spill buffers (bf16 to halve HBM traffic); q and v stay
        # SBUF-resident (qres/vres below), only k needs the DRAM partition
        # regroup
        ka2 = dpool.tile([H, E2, T], bf16, tag="ka2")
        kvinA = dpool.tile([8, E2, M], bf16, tag="kvinA")
        kvoutA = dpool.tile([8, E2, M], bf16, tag="kvoutA")
        kvinB = dpool.tile([8, E2, M], bf16, tag="kvinB")
        kvoutB = dpool.tile([8, E2, M], bf16, tag="kvoutB")

        def emit_body(rep, phases):
          with ExitStack() as rctx:
              # program-lifetime constants (cpool; reused across reps)
              idt = cpool.tile([128, 128], bf16, tag="idt")
              nc.sync.dma_start(idt[:], ident[:])
              omg = cpool.tile([128, M], bf16, tag="omg")
              nc.sync.dma_start(omg[:], omegx[:])
              omgq_t = cpool.tile([128, M], bf16, tag="omgq")
              nc.sync.dma_start(omgq_t[:], omegq[:])
              lmt = cpool.tile([E2, 128], bf16, tag="lmt")
              nc.sync.dma_start(lmt[:], lmT[:])
              nlmt2 = cpool.tile([128, 1], f32, tag="nlmt2")
              nc.sync.dma_start(nlmt2[:], nlm2[:])
              nscalt = cpool.tile([128, 1], f32, tag="nscalt")
              nc.sync.dma_start(nscalt[:], nscal[:])
              swpt = cpool.tile([128, 128], bf16, tag="swpt")
              nc.sync.dma_start(swpt[:], swp[:])
              outbt = cpool.tile([128, 8], f32, tag="outbt")
              nc.sync.dma_start(outbt[:], outb[:])
              qcost = cpool.tile([128, T], bf16, tag="qcost")
              nc.sync.dma_start(qcost[:], qcos[:])
              qsint = cpool.tile([128, T], bf16, tag="qsint")
              nc.sync.dma_start(qsint[:], qsin[:])
              # block-ones stationaries for per-head |k|^2 partition
              # reduction: column 2cc(+1) of variant cc selects head 2cc(+1)
              ones16 = cpool.tile([128, 8, 16], f32r, tag="ones16")
              nc.vector.memset(ones16[:].bitcast(f32), 0.0)
              for _cc in range(8):
                  nc.vector.memset(
                      ones16[0:64, _cc, 2 * _cc:2 * _cc + 1].bitcast(f32), 1.0)
                  nc.vector.memset(
                      ones16[64:128, _cc, 2 * _cc + 1:2 * _cc + 2].bitcast(f32),
                      1.0)
              # SBUF-resident front-end outputs (skip the DRAM roundtrip):
              # q channel-major [128, cc, T] and v token-major [128, tcn, H, E2]
              respool = rctx.enter_context(tc.tile_pool(name="res", bufs=1))
              qres = respool.tile([128, 8, T], bf16, tag="qres")
              vres = respool.tile([128, 8, H, E2], bf16, tag="vres")
              # augmented v columns: [ones, zero] (constant per rep)
              nc.vector.memset(vres[:, :, :, DH:DH + 1], 1.0)
              nc.vector.memset(vres[:, :, :, DH + 1:E2], 0.0)
              # fe/kside pools live in fectx so they free before the q side
              # (closed explicitly after the q front end)
              fectx = rctx.enter_context(ExitStack())
              # x is shared by both front-end passes; loaded once per rep
              xpool = fectx.enter_context(tc.tile_pool(name="xp", bufs=1))
              xt = xpool.tile([128, 8, T + 2], f32r, tag="xt")
              nc.sync.dma_start(
                  xt[:], xT[:].rearrange("(a p) n -> p a n", p=128))

              # ============ front end (two 512-token halves) ============
              # pools are shared by both passes and stay open across the k
              # side so the scheduler can overlap k-side (ACT-heavy) with
              # the q front end (PE-heavy)
              fpools = {}
              fpools["h"] = fectx.enter_context(tc.tile_pool(name="hp", bufs=2))
              fpools["f"] = fectx.enter_context(tc.tile_pool(name="fp", bufs=2))
              fpools["wc"] = fectx.enter_context(tc.tile_pool(name="wc", bufs=3))
              fpools["wr"] = fectx.enter_context(tc.tile_pool(name="wr", bufs=3))
              fpools["wp"] = fectx.enter_context(tc.tile_pool(name="wp", bufs=1))
              fpools["tb"] = fectx.enter_context(tc.tile_pool(name="tb", bufs=1))
              fpools["scr"] = fectx.enter_context(tc.tile_pool(name="scr", bufs=3))
              fpools["rope"] = fectx.enter_context(tc.tile_pool(name="rope", bufs=3))
              fpools["out"] = fectx.enter_context(tc.tile_pool(name="feo", bufs=2))
              psfe = fectx.enter_context(
                  tc.tile_pool(name="psfe", bufs=2, space="PSUM"))

              cbt = fpools["tb"].tile([128, 3, 8], f32, tag="cbt")
              nc.sync.dma_start(cbt[:], cb[:])
              rffb2t = fpools["tb"].tile([128, 3, G, 2], f32, tag="rffb2t")
              nc.sync.dma_start(rffb2t[:], rffb2[:])
              rffbrt = fpools["tb"].tile([128, 3, G, 2], f32, tag="rffbrt")
              nc.sync.dma_start(rffbrt[:], rffbr[:])

              def fe_pass(i_list):
                with ExitStack() as fe:
                  # pass-local PSUM pools (v/k extras) so banks free between
                  # passes
                  if 2 in i_list:
                      pk2 = fe.enter_context(
                          tc.tile_pool(name="pk2", bufs=2, space="PSUM"))
                  if 1 in i_list:
                      pknp = fe.enter_context(
                          tc.tile_pool(name="pkn", bufs=1, space="PSUM"))

                  wrt_hold = {}
                  for i in i_list:
                      wct = [fpools["wc"].tile([128, 2, C], f32r, tag="wconv",
                                               name=f"wct{i}_{t_}_r{rep}")
                             for t_ in range(3)]
                      for tap in range(3):
                          nc.sync.dma_start(
                              wct[tap][:],
                              convw[i, tap].rearrange("(a p) n -> p a n", p=128))
                      wpt = fpools["wp"].tile([128, 6, C], bf16, tag="wproj")
                      nc.sync.dma_start(
                          wpt[:], projw[i].rearrange("(a p) n -> p a n", p=128))
                      hTs, featss = [], []
                      for hf in range(2):
                          t0 = hf * HF
                          # ---- grouped conv -> hT half [C, 512] cm ----
                          hT = fpools["h"].tile([128, 8, HF], f32r, tag="hT",
                                                name=f"hT{i}_{hf}_r{rep}")
                          hTs.append(hT)
                          for coc in range(8):
                              g = coc // 2
                              pc = psfe.tile([128, HF], f32, tag="pfe")
                              n = 0
                              for tap in range(3):
                                  for cic in range(2):
                                      nc.tensor.matmul(
                                          pc[:],
                                          wct[tap][:, cic, coc * 128:(coc + 1) * 128],
                                          xt[:, g * 2 + cic, t0 + tap:t0 + tap + HF],
                                          start=(n == 0), stop=(n == 5))
                                      n += 1
                              if i == 0:
                                  # during the k-side overlap window ACT is
                                  # saturated; evacuate on DVE instead
                                  nc.vector.tensor_scalar_add(
                                      hT[:, coc, :], pc[:],
                                      cbt[:, i, coc:coc + 1])
                              else:
                                  nc.scalar.activation(
                                      hT[:, coc, :], pc[:], AF.Identity,
                                      bias=cbt[:, i, coc:coc + 1])
                      for hf in range(2):
                          t0 = hf * HF
                          hT = hTs[hf]
                          # ---- RFF ----
                          feats = fpools["f"].tile([128, 6, HF], bf16, tag="feats",
                                                   name=f"feats{i}_{hf}_r{rep}")
                          featss.append(feats)
                          for g in range(G):
                              if hf == 0:
                                  wrt = fpools["wr"].tile([128, 8, MS], f32r,
                                                          tag="wrff",
                                                          name=f"wrt{i}_{g}_r{rep}")
                                  nc.sync.dma_start(
                                      wrt[:],
                                      rffw[i, g].rearrange("(a p) m -> p a m", p=128))
                                  wrt_hold[g] = wrt
                              else:
                                  wrt = wrt_hold[g]
                              for mc in range(2):
                                  pr = psfe.tile([128, HF], f32, tag="pfe")
                                  for cc in range(8):
                                      nc.tensor.matmul(
                                          pr[:],
                                          wrt[:, cc, mc * 128:(mc + 1) * 128],
                                          hT[:, cc, :],
                                          start=(cc == 0), stop=(cc == 7))
                                  # cos(a) = sin(2pi*(y - k) + b) with the
                                  # sgam/2pi scale folded into wrt on the host:
                                  # pr is already in period units. k = round(pr
                                  # + beta) via the magic-number trick; any
                                  # nearby integer works (sin periodicity), the
                                  # per-partition phase rides in the Sin bias.
                                  t1 = fpools["scr"].tile([128, HF], f32, tag="t1")
                                  nc.vector.tensor_scalar(
                                      t1[:], pr[:], rffb2t[:, i, g, mc:mc + 1],
                                      MAGIC, op0=ALU.add, op1=ALU.add)
                                  u = fpools["scr"].tile([128, HF], f32, tag="u")
                                  nc.vector.scalar_tensor_tensor(
                                      u[:], t1[:], MAGIC, pr[:],
                                      op0=ALU.subtract, op1=ALU.subtract)
                                  nc.scalar.activation(feats[:, g * 2 + mc, :], u[:],
                                                       AF.Sin, scale=-TWO_PI,
                                                       bias=rffbrt[:, i, g, mc:mc + 1])
                      for hf in range(2):
                          t0 = hf * HF
                          feats = featss[hf]
                          # ---- dense proj ----
                          if i == 0:
                              # q channel-major + RoPE. sin/cos rows repeat at
                              # d and d+32, so swp(q*sin) = swp(q)*sin: the
                              # PSUM evacuation fuses into the sin/cos muls on
                              # Pool and no ACT op is needed at all.
                              for cc in range(8):
                                  pq = psfe.tile([128, HF], f32, tag="pfe")
                                  for fc in range(6):
                                      nc.tensor.matmul(
                                          pq[:],
                                          wpt[:, fc, cc * 128:(cc + 1) * 128],
                                          feats[:, fc, :],
                                          start=(fc == 0), stop=(fc == 5))
                                  qs = fpools["rope"].tile([128, HF], bf16,
                                                           tag="rsn")
                                  nc.vector.tensor_mul(qs[:], pq[:],
                                                       qsint[:, t0:t0 + HF])
                                  qc = fpools["rope"].tile([128, HF], bf16,
                                                           tag="rcs")
                                  nc.vector.tensor_mul(qc[:], pq[:],
                                                       qcost[:, t0:t0 + HF])
                                  psw = psfe.tile([128, HF], f32, tag="pfe")
                                  nc.tensor.matmul(psw[:], swpt[:], qs[:],
                                                   start=True, stop=True)
                                  nc.vector.tensor_add(
                                      qres[:, cc, t0:t0 + HF], qc[:], psw[:])
                          elif i == 1:
                              # k channel-major + RoPE (norm-preserving), with
                              # per-head |k|^2 row; spills straight into the
                              # nystrom-ready [E2, T] per-head layout
                              pknall = pknp.tile([16, HF], f32, tag="pknt")
                              kfl = {}
                              for cc in range(9):
                                  # software-pipelined: chunk cc's matmuls run
                                  # while chunk cc-1's RoPE chain drains
                                  if cc < 8:
                                      pq = psfe.tile([128, HF], f32, tag="pfe")
                                      for fc in range(6):
                                          nc.tensor.matmul(
                                              pq[:],
                                              wpt[:, fc, cc * 128:(cc + 1) * 128],
                                              feats[:, fc, :],
                                              start=(fc == 0), stop=(fc == 5))
                                      # |k|^2 pre-RoPE (rotation preserves
                                      # norm), squared straight out of PSUM on
                                      # ACT (one PSUM operand allowed per op)
                                      ksq = fpools["scr"].tile([128, HF], f32r,
                                                               tag="u")
                                      nc.scalar.activation(ksq[:], pq[:],
                                                           AF.Square)
                                      ks = fpools["rope"].tile([128, HF], bf16,
                                                               tag="rsn")
                                      nc.vector.tensor_mul(ks[:], pq[:],
                                                           qsint[:, t0:t0 + HF])
                                      kc = fpools["rope"].tile([128, HF], bf16,
                                                               tag="rcs")
                                      nc.vector.tensor_mul(kc[:], pq[:],
                                                           qcost[:, t0:t0 + HF])
                                      kfl[cc] = (ks, kc, ksq)
                                  if cc >= 1:
                                      c0 = cc - 1
                                      ks, kc, ksq = kfl.pop(c0)
                                      nc.tensor.matmul(pknall[:], ones16[:, c0, :],
                                                       ksq[:], start=(c0 == 0),
                                                       stop=(c0 == 7))
                                      psw = psfe.tile([128, HF], f32, tag="pfe")
                                      nc.tensor.matmul(psw[:], swpt[:], ks[:],
                                                       start=True, stop=True)
                                      kro = fpools["out"].tile([128, HF], bf16,
                                                               tag="qro")
                                      nc.vector.tensor_add(kro[:], kc[:], psw[:])
                                      nc.sync.dma_start(
                                          ka2[2 * c0, 0:DH, t0:t0 + HF],
                                          kro[0:64, :])
                                      nc.sync.dma_start(
                                          ka2[2 * c0 + 1, 0:DH, t0:t0 + HF],
                                          kro[64:128, :])
                              # rows DH..E2 of ka2: [|k|^2, 0] (row 65 must be
                              # finite: lmt row 65 is 0 but NaN*0 = NaN)
                              k2sb = fpools["tb"].tile([16, 2, HF], bf16,
                                                       tag="k2all",
                                                       name=f"k2a_{hf}_r{rep}")
                              nc.scalar.activation(k2sb[:, 0, :], pknall[:],
                                                   AF.Copy)
                              nc.vector.memset(k2sb[:, 1, :], 0.0)
                              nc.sync.dma_start(
                                  ka2[:, DH:E2, t0:t0 + HF], k2sb[:])
                          else:
                              # v token-major, written straight into the
                              # SBUF-resident vres (one bank [128,512] psum
                              # per channel half)
                              for tl in range(4):
                                  tcn = hf * 4 + tl
                                  for p in range(2):
                                      pk = pk2.tile([128, HF], f32, tag="pk")
                                      for fc in range(6):
                                          nc.tensor.matmul(
                                              pk[:],
                                              feats[:, fc,
                                                    tl * 128:(tl + 1) * 128],
                                              wpt[:, fc, p * 512:(p + 1) * 512],
                                              start=(fc == 0), stop=(fc == 5))
                                      pkv = pk[:].rearrange(
                                          "p (h d) -> p h d", d=DH)
                                      nc.scalar.activation(
                                          vres[:, tcn, p * 8:(p + 1) * 8, 0:DH],
                                          pkv, AF.Copy)

              fe_pass([1, 2])

              # ================= k side: nystrom + FAVOR + kv =================
              # pools on fectx: they coexist with the fe pools so the
              # scheduler can overlap the k side with the q front end below
              if "k" in phases:
                  kvp = fectx.enter_context(tc.tile_pool(name="kvld", bufs=2))
                  attp = fectx.enter_context(tc.tile_pool(name="attk", bufs=2))
                  kpp = fectx.enter_context(tc.tile_pool(name="kps", bufs=2))
                  smlk = fectx.enter_context(tc.tile_pool(name="smlk", bufs=2))
                  pNp = fectx.enter_context(tc.tile_pool(name="pN", bufs=2,
                                                         space="PSUM"))
                  pFp = fectx.enter_context(tc.tile_pool(name="pF", bufs=2,
                                                         space="PSUM"))
                  pKVp = fectx.enter_context(tc.tile_pool(name="pKV", bufs=2,
                                                          space="PSUM"))
                  for h in range(H):
                      kvin = kvinA if h < 8 else kvinB
                      krt = kvp.tile([E2, T], bf16, tag="krT")
                      nc.sync.dma_start(krt[:], ka2[h])
                      knxs = []
                      for p in range(2):
                          pn = pNp.tile([128, HF], f32, tag="pN")
                          nc.tensor.matmul(
                              pn[:], lmt[:], krt[:, p * 512:(p + 1) * 512],
                              start=True, stop=True)
                          # rows 0-63 get kn = exp(P/32 - nl/64); rows 64-127
                          # (same P via duplicated landmark columns) get kn^2
                          knx = attp.tile([128, HF], bf16, tag="knx",
                                          name=f"knx{h}_{p}_r{rep}")
                          nc.scalar.activation(knx[:], pn[:], AF.Exp,
                                               bias=nlmt2[:], scale=nscalt[:])
                          knxs.append(knx)
                      pkv_ps = pKVp.tile([E2, M], f32, tag="pKV")
                      kps = {}
                      for step in range(5):
                          # token chunks processed in pairs so the FAVOR exp
                          # runs as one [128, 512] ACT op
                          if step < 4:
                              pr2 = step
                              pf = pFp.tile([128, 2, M], f32, tag="pF")
                              for j in range(2):
                                  c = 2 * pr2 + j
                                  nc.tensor.matmul(
                                      pf[:, j, :],
                                      knxs[c // 4][:,
                                                   (c % 4) * 128:(c % 4 + 1) * 128],
                                      omg[:], start=True, stop=True)
                              kpt = kpp.tile([128, 2, M], bf16, tag="kp",
                                             name=f"kp{h}_{pr2}_r{rep}")
                              nc.scalar.activation(kpt[:], pf[:], AF.Exp)
                              kps[pr2] = kpt
                          if step >= 1:
                              pr2 = step - 1
                              kpt = kps.pop(pr2)
                              for j in range(2):
                                  c = 2 * pr2 + j
                                  nc.tensor.matmul(pkv_ps[:], vres[:, c, h, :],
                                                   kpt[:, j, :],
                                                   start=(c == 0), stop=(c == 7))
                      kvsb = smlk.tile([E2, M], bf16, tag="kvsb")
                      nc.vector.tensor_copy(kvsb[:], pkv_ps[:])
                      nc.sync.dma_start(kvin[h % 8], kvsb[:])
                      if h == 15:
                          if collectives:
                              nc.gpsimd.collective_compute(
                                  "AllReduce", mybir.AluOpType.add,
                                  replica_groups=rg,
                                  ins=[kvinB.opt()], outs=[kvoutB.opt()])
                          else:
                              nc.sync.dma_start(kvoutB[:], kvinB[:])
                      if h == 7:
                          if collectives:
                              nc.gpsimd.collective_compute(
                                  "AllReduce", mybir.AluOpType.add,
                                  replica_groups=rg,
                                  ins=[kvinA.opt()], outs=[kvoutA.opt()])
                          else:
                              nc.sync.dma_start(kvoutA[:], kvinA[:])

              # q front end: emitted after the k side but overlaps it on the
              # PE (disjoint pools, dependency-driven scheduling); also hides
              # the kv AllReduces
              fe_pass([0])
              # free fe + k-side SBUF/PSUM before the q side allocates
              fectx.close()

              # prefetch output dense weights (first use in phase O)
              wop = rctx.enter_context(tc.tile_pool(name="wo", bufs=1))
              wot = wop.tile([128, 8, C], bf16, tag="wout")
              nc.sync.dma_start(
                  wot[:], outw[:].rearrange("(a p) n -> p a n", p=128))

              # ================= q side: FAVOR + num/denom =================
              opool = rctx.enter_context(tc.tile_pool(name="oc", bufs=1))
              ocm = opool.tile([128, 8, T], bf16, tag="ocm")
              if "q" in phases:
                with ExitStack() as qc:
                  attq = qc.enter_context(tc.tile_pool(name="attq", bufs=3))
                  smlq = qc.enter_context(tc.tile_pool(name="smlq", bufs=2))
                  pQ2p = qc.enter_context(tc.tile_pool(name="pQ2", bufs=2,
                                                       space="PSUM"))
                  pNump = qc.enter_context(tc.tile_pool(name="pNum", bufs=2,
                                                        space="PSUM"))
                  pTqp = qc.enter_context(tc.tile_pool(name="pTq", bufs=2,
                                                       space="PSUM"))
                  fronts = {}
                  for step in range(H + 1):
                    if step < H:
                      h = step
                      kvout = kvoutA if h < 8 else kvoutB
                      kvs = attq.tile([E2, M], bf16, tag="kvs", name=f"kvs{h}_r{rep}")
                      nc.sync.dma_start(kvs[:], kvout[h % 8])
                      kvf = attq.tile([128, 2, E2], bf16, tag="kvf",
                                      name=f"kvf{h}_r{rep}")
                      for mc in range(2):
                          ptq = pTqp.tile([128, E2], bf16, tag="pTq")
                          nc.tensor.transpose(ptq[:], kvs[:, mc * 128:(mc + 1) * 128],
                                              idt[0:E2, 0:E2])
                          nc.vector.tensor_copy(kvf[:, mc, :], ptq[:])
                      qpt = attq.tile([128, 2, T], bf16, tag="qp", name=f"qpt{h}_r{rep}")
                      hb = (h % 2) * 64
                      for mc in range(2):
                          for p in range(2):
                              pq2 = pQ2p.tile([128, HF], f32, tag="pQ2")
                              nc.tensor.matmul(
                                  pq2[:],
                                  omgq_t[hb:hb + 64, mc * 128:(mc + 1) * 128],
                                  qres[hb:hb + 64, h // 2,
                                       p * 512:(p + 1) * 512],
                                  start=True, stop=True)
                              nc.scalar.activation(
                                  qpt[:, mc, p * 512:(p + 1) * 512], pq2[:], AF.Exp)
                      fronts[h] = (kvf, qpt)
                    if step >= 1:
                      h = step - 1
                      hb = (h % 2) * 64
                      kvf, qpt = fronts.pop(h)
                      pnum = pNump.tile([E2, T], f32, tag="pNum")
                      for mc in range(2):
                          for p in range(2):
                              nc.tensor.matmul(
                                  pnum[:, p * 512:(p + 1) * 512],
                                  kvf[:, mc, :],
                                  qpt[:, mc, p * 512:(p + 1) * 512],
                                  start=(mc == 0), stop=(mc == 1))
                      # evacuate psum immediately so pNum recycles without
                      # waiting on the recip/broadcast/mul consumer chain
                      nsb = smlq.tile([E2, T], f32, tag="nsb",
                                      name=f"nsb{h}_r{rep}")
                      nc.vector.tensor_copy(nsb[:], pnum[:])
                      drow = smlq.tile([E2, T], f32, tag="drow")
                      nc.vector.reciprocal(drow[64:65, :], nsb[64:65, :])
                      rc = smlq.tile([1, T], f32, tag="rcp")
                      nc.sync.dma_start(rc[0:1, :], drow[64:65, :])
                      rb = smlq.tile([64, T], f32, tag="rb")
                      nc.gpsimd.partition_broadcast(rb[:], rc[0:1, :])
                      if hb == 0:
                          nc.vector.tensor_mul(ocm[0:64, h // 2, :],
                                               nsb[0:64, :], rb[:])
                      else:
                          osc = smlq.tile([64, T], bf16, tag="osc")
                          nc.vector.tensor_mul(osc[:], nsb[0:64, :], rb[:])
                          nc.sync.dma_start(ocm[64:128, h // 2, :], osc[:])

              # ================= output dense =================
              if "o" in phases:
                with (
                  tc.tile_pool(name="psO", bufs=3, space="PSUM") as psO,
                  tc.tile_pool(name="oto", bufs=2) as otop,
                ):
                  for coc in range(8):
                      po = psO.tile([128, T], f32, tag="pO")
                      for p in range(2):
                          for cc in range(8):
                              nc.tensor.matmul(
                                  po[:, p * 512:(p + 1) * 512],
                                  wot[:, cc, coc * 128:(coc + 1) * 128],
                                  ocm[:, cc, p * 512:(p + 1) * 512],
                                  start=(cc == 0), stop=(cc == 7))
                      ot = otop.tile([128, T], f32, tag="ot")
                      nc.scalar.activation(ot[:], po[:], AF.Identity,
                                           bias=outbt[:, coc:coc + 1])
                      nc.sync.dma_start(outT[coc * 128:(coc + 1) * 128, :], ot[:])

        for _rep in range(repeat):
            emit_body(_rep, phases)
    nc.compile()
    return nc


def _host_prep(x, conv_k, conv_b, rff_w, rff_b, proj_w, proj_b, omega,
               landmarks, out_w, out_b):
    """Shared + per-core input arrays (f32 where phase-sensitive, else bf16)."""
    import ml_dtypes
    f32 = np.float32
    bf16 = ml_dtypes.bfloat16
    assert not np.any(proj_b), "kernel assumes proj_b == 0 (spec: zeros)"
    S = x.shape[1]

    # rope tables in fp32 arithmetic to match the jax fp32 reference
    inv = (1.0 / (10000.0 ** (np.arange(0, DH, 2, dtype=f32) / f32(DH)))).astype(f32)
    fmat = np.arange(S, dtype=f32)[:, None] * inv[None, :]
    emb = np.concatenate([fmat, fmat], axis=1).astype(f32)     # [S, 64]
    sin_t, cos_t = np.sin(emb), np.cos(emb)

    shared = {
        "convw": np.ascontiguousarray(conv_k, f32),
        "cb": np.ascontiguousarray(
            conv_b.reshape(3, 8, 128).transpose(2, 0, 1), f32),
        # sqrt(2*gamma)/2pi folded into the RFF weights: the PE emits phase in
        # period units directly
        "rffw": np.ascontiguousarray(
            rff_w * (np.sqrt(2.0 * np.array([0.5, 1.0, 2.0], np.float64))
                     / (2.0 * np.pi))[None, :, None, None], f32),
        "rffb2": np.ascontiguousarray(
            (rff_b / (2.0 * np.pi) + 0.25).reshape(3, G, 2, 128)
            .transpose(3, 0, 1, 2), f32),
        "rffbr": np.ascontiguousarray(
            (rff_b + 0.5 * np.pi).reshape(3, G, 2, 128)
            .transpose(3, 0, 1, 2), f32),
        "projw": np.ascontiguousarray(proj_w * np.sqrt(2.0 / MS), bf16),
        "outw": np.ascontiguousarray(out_w, bf16),
        "outb": np.ascontiguousarray(out_b.reshape(8, 128).T, f32),
        "ident": np.eye(128, dtype=bf16),
    }
    nl = (landmarks.astype(f32) ** 2).sum(1)[:, None] / f32(DH)
    shared["nlm2"] = np.concatenate([-nl, -2.0 * nl], 0).astype(f32)
    shared["nscal"] = np.concatenate(
        [np.full((64, 1), 1.0 / 32.0, f32), np.full((64, 1), 2.0 / 32.0, f32)], 0)
    swp = np.zeros((128, 128), f32)
    for blk in range(2):
        for d in range(32):
            swp[blk * 64 + d + 32, blk * 64 + d] = -1.0
            swp[blk * 64 + d, blk * 64 + d + 32] = 1.0
    shared["swp"] = swp.astype(bf16)
    omegx = np.full((128, M), -0.5, f32)
    omegx[0:DH, 0:M] = omega
    shared["omegx"] = omegx.astype(bf16)
    shared["omegq"] = np.ascontiguousarray(
        np.concatenate([omega, omega], axis=0), bf16)
    lmTa = np.zeros((E2, DH), f32)
    lmTa[0:DH] = landmarks.T
    lmTa[DH] = -0.5
    shared["lmT"] = np.ascontiguousarray(
        np.concatenate([lmTa, lmTa], axis=1), bf16)

    per_core = []
    for j in range(NCORES):
        b, s0 = j // 4, (j % 4) * T
        xp = np.pad(x[b], ((1, 1), (0, 0)))
        m = dict(shared)
        m["xT"] = np.ascontiguousarray(xp[s0:s0 + T + 2].T, f32)
        m["qcos"] = np.ascontiguousarray(
            np.tile(cos_t[s0:s0 + T].T, (2, 1)), bf16)
        # unsigned: the rotate-half signs live in the swp permutation matrix
        m["qsin"] = np.ascontiguousarray(
            np.tile(sin_t[s0:s0 + T].T, (2, 1)), bf16)
        per_core.append(m)
    return per_core


def kernel(x, conv_k, conv_b, rff_w, rff_b, proj_w, proj_b, omega, landmarks,
           out_w, out_b):
    from concourse.bass_utils import run_bass_kernel_spmd

    if "nc" not in _CACHE:
        _CACHE["nc"] = _build(NCORES)
    nc = _CACHE["nc"]
    in_maps = _host_prep(
        np.asarray(x, np.float32), np.asarray(conv_k, np.float32),
        np.asarray(conv_b, np.float32), np.asarray(rff_w, np.float32),
        np.asarray(rff_b, np.float32), np.asarray(proj_w, np.float32),
        np.asarray(proj_b, np.float32), np.asarray(omega, np.float32),
        np.asarray(landmarks, np.float32), np.asarray(out_w, np.float32),
        np.asarray(out_b, np.float32))
    res = run_bass_kernel_spmd(nc, in_maps, core_ids=list(range(NCORES)))
    out = np.empty((2, 4096, C), np.float32)
    for j in range(NCORES):
        b, s0 = j // 4, (j % 4) * T
        out[b, s0:s0 + T] = res.results[j]["outT"].T
    return out

